# revision 24
# baseline (speedup 1.0000x reference)
"""Trainium2 Bass kernel for the sparse_attention PoC block.

Reference computation (per batch item):
  qkv = x @ qkv_w.T            [N, 3C] -> q,k,v heads [H, N, D]
  attn = (q @ k.T) * scale     [H, N, N]
  block edits: attn[:S1, S2:] = attn[:S1, S1:S2] (pre-bias copy), then
  -100 bias on [:S1, S1:S2], [S1:S2, S2:], [S2:, S1:S2]; softmax;
  attn @ v; proj.

Distribution: pure data-parallel over batch B=64 across 8 NeuronCores
(8 batch items per core, weights replicated). No collectives.

Layout strategy per core (TensorE matmuls in bf16, fp32 PSUM accum):
  - x transposed on TensorE to xT [C, N]
  - q,k computed per head in transposed orientation psum[d=96, N] =
    W_head^T.T @ xT into q_all/k_all [128, H, N] (rows 0:96 = head data,
    rows 96:128 = bias extension rows); v in natural orientation [N, 768]
  - scores computed transposed sT[j=keys, q=queries] = k_ext.T @ q_ext in
    ONE matmul per key tile: the -100 block edits ride as rank-2 updates in
    contraction rows 96/97 (key-group indicators on the k side, query-group
    bias patterns on the q side); the pre-bias "copy" edit is realized by
    overwriting kT's aux-slot columns with the lang key vectors (free-dim
    copy), with a tiny 20x20 correction matmul restoring the true aux x aux
    block (suppressed in the main tile by the rank-2 bias)
  - softmax without max-subtraction (logits are O(1); suppressed entries
    underflow exp to ~0 exactly as the reference's -100 bias does); exp on
    ScalarE with the 1/sqrt(D) scale folded in
  - attn@v with a fused [v | ones] stationary column gives unnormalized
    oT [D+1, q] + denominator row; normalize via fast approx reciprocal +
    gpsimd partition_broadcast, into aoT [96, H, N]
  - proj psum[n, oc] = aoT.T @ proj_w^T (K=96 per head), + proj_b, DMA out

Partition-alignment rule (walrus verifier): compute-engine access patterns
must start at partition 0/32/64/96 (max 128/32/64/32 partitions); matmul
operands must start at partition 0. Misaligned extractions (v_aux at rows
88:108) go through DMA, which has no such restriction.
"""

import numpy as np

B, N, C = 64, 236, 768
H, D = 8, 96
S1, S2 = 196, 216
BIAS = 100.0
SCALE = D ** -0.5
BIAS_RAW = BIAS / SCALE  # applied on raw (pre-scale) scores

N_CORES = 8
B_LOC = B // N_CORES

NT = [(0, 128), (128, 108)]  # token tiles (partition dim) / key tiles
NC_CH = C // 128  # 6 contraction chunks over C
KEXT = 128  # contraction size for scores: 96 head dims + bias rows


def part_cap(s):
    return 128 if s == 0 else 64 if s == 64 else 32


def part_pieces2(s1, s2, size):
    """Split a partition-range copy (dest start s1, src start s2, length
    size) into pieces legal for compute engines on both sides."""
    out = []
    off = 0
    while off < size:
        take = min(size - off, part_cap((s1 + off) % 128),
                   part_cap((s2 + off) % 128))
        out.append((s1 + off, s2 + off, take))
        off += take
    return out


def head_fragments(o_lo, o_hi, base):
    """Split channel range [o_lo, o_hi) (relative to `base`) at head
    boundaries (96) and legal partition pieces. Yields
    (head, d_lo, d_hi, p_lo, p_hi) with p relative to o_lo."""
    frags = []
    g = o_lo
    while g < o_hi:
        h = (g - base) // D
        d_lo = (g - base) - h * D
        take = min(o_hi - g, D - d_lo)
        for (d0, p0, sz) in part_pieces2(d_lo, g - o_lo, take):
            frags.append((h, d0, d0 + sz, p0, p0 + sz))
        g += take
    return frags


def build(b_loc=B_LOC):
    import concourse.bass as bass  # noqa: F401
    import concourse.tile as tile
    import concourse.bacc as bacc
    from concourse import mybir
    from concourse.masks import make_identity

    f32 = mybir.dt.float32
    bf16 = mybir.dt.bfloat16
    AF = mybir.ActivationFunctionType
    OP = mybir.AluOpType

    nc = bacc.Bacc("TRN2", target_bir_lowering=False)
    x_d = nc.dram_tensor("x", [b_loc, N, C], f32, kind="ExternalInput")
    qkvw_d = nc.dram_tensor("qkv_w", [3 * C, C], f32, kind="ExternalInput")
    projw_d = nc.dram_tensor("proj_w", [C, C], f32, kind="ExternalInput")
    projb_d = nc.dram_tensor("proj_b", [C], f32, kind="ExternalInput")
    out_d = nc.dram_tensor("out", [b_loc, N, C], f32, kind="ExternalOutput")

    with tile.TileContext(nc) as tc:
        with (
            tc.tile_pool(name="const", bufs=1) as constp,
            tc.tile_pool(name="wload", bufs=3) as wloadp,
            tc.tile_pool(name="xload", bufs=2) as xloadp,
            tc.tile_pool(name="xt", bufs=2) as xtp,
            tc.tile_pool(name="qk", bufs=2) as qkp,
            tc.tile_pool(name="vsb", bufs=2) as vsbp,
            tc.tile_pool(name="psb", bufs=4) as psbp,
            tc.tile_pool(name="ao", bufs=2) as aop,
            tc.tile_pool(name="osb", bufs=2) as osbp,
            tc.tile_pool(name="tiny", bufs=6) as tinyp,
            tc.tile_pool(name="ps_mm", bufs=3, space="PSUM") as ps_mm,
            tc.tile_pool(name="ps_s", bufs=3, space="PSUM") as ps_s,
            tc.tile_pool(name="ps_o", bufs=2, space="PSUM") as ps_o,
        ):
            prep_pools = [(ps_mm, "mm"), (ps_s, "s"), (ps_o, "o")]
            # ---------------- constants ----------------
            ident = constp.tile([128, 128], bf16)
            make_identity(nc, ident[:])

            # Bias-extension master rows (contraction rows 96:128).
            # wmaster (q side): row0 = w1[q] = -BIAS_RAW on img+aux queries;
            #                   row1 = w2[q] = -BIAS_RAW on lang+aux queries.
            # umaster (k side): row0 = u1[j] = 1 on lang key slots;
            #                   row1 = u2[j] = 1 on aux key slots.
            wmaster = constp.tile([32, N], bf16)
            umaster = constp.tile([32, N], bf16)
            nc.vector.memset(wmaster[:], 0.0)
            nc.vector.memset(umaster[:], 0.0)
            nc.vector.memset(wmaster[0:1, 0:S1], -BIAS_RAW)
            nc.vector.memset(wmaster[0:1, S2:N], -BIAS_RAW)
            nc.vector.memset(umaster[0:1, S1:S2], 1.0)
            # row 1 of each master: build in a [1, N] stage, DMA to row 1
            # (compute engines cannot address partition 1; DMA can).
            w2row = constp.tile([1, N], bf16)
            nc.vector.memset(w2row[:], 0.0)
            nc.vector.memset(w2row[0:1, S1:N], -BIAS_RAW)
            u2row = constp.tile([1, N], bf16)
            nc.vector.memset(u2row[:], 0.0)
            nc.vector.memset(u2row[0:1, S2:N], 1.0)
            nc.sync.dma_start(wmaster[1:2, :], w2row[:])
            nc.sync.dma_start(umaster[1:2, :], u2row[:])

            # ---------------- weights prep ----------------
            qkvwT = [constp.tile([128, 3 * C], bf16, name=f"qkvwT{i}")
                     for i in range(NC_CH)]
            # proj_w^T stored per head: projwTh[h] = proj_w.T[96h:96h+96, :]
            projwTh = [constp.tile([96, C], bf16, name=f"projwTh{h}")
                       for h in range(H)]

            for r in range(3 * C // 128):  # 18 row-chunks of qkv_w
                wl = wloadp.tile([128, C], f32, tag="wl")
                nc.sync.dma_start(wl[:], qkvw_d[r * 128:(r + 1) * 128, :])
                wb = wloadp.tile([128, C], bf16, tag="wb")
                nc.vector.tensor_copy(wb[:], wl[:])
                for ci in range(NC_CH):
                    pool_i, tag_i = prep_pools[(r * NC_CH + ci) % 3]
                    pt = pool_i.tile([128, 128], bf16, tag=tag_i)
                    nc.tensor.transpose(
                        pt[:], wb[:, ci * 128:(ci + 1) * 128], ident[:])
                    nc.any.tensor_copy(
                        qkvwT[ci][:, r * 128:(r + 1) * 128], pt[:])
            for r in range(C // 128):  # 6 row-chunks of proj_w
                wl = wloadp.tile([128, C], f32, tag="wl")
                nc.sync.dma_start(wl[:], projw_d[r * 128:(r + 1) * 128, :])
                wb = wloadp.tile([128, C], bf16, tag="wb")
                nc.vector.tensor_copy(wb[:], wl[:])
                for h in range(H):
                    pool_i, tag_i = prep_pools[(r * H + h) % 3]
                    pt = pool_i.tile([128, 128], bf16, tag=tag_i)
                    nc.tensor.transpose(
                        pt[:96, :], wb[:, h * D:(h + 1) * D], ident[:])
                    nc.any.tensor_copy(
                        projwTh[h][:, r * 128:(r + 1) * 128], pt[:96, :])

            # proj_b broadcast to [128, C] via gpsimd partition_broadcast
            pb_row = constp.tile([1, C], f32)
            nc.sync.dma_start(pb_row[:], projb_d[None, :])
            pb_bcast = constp.tile([128, C], f32)
            nc.gpsimd.partition_broadcast(pb_bcast[:], pb_row[:])

            # ---------------- per-batch ----------------
            for b in range(b_loc):
                # load + cast + transpose x
                xT = [xtp.tile([128, N], bf16, name=f"xT{ci}")
                      for ci in range(NC_CH)]
                for nt, (noff, nsz) in enumerate(NT):
                    xf = xloadp.tile([128, C], f32, tag="xf")
                    nc.sync.dma_start(xf[:nsz], x_d[b, noff:noff + nsz, :])
                    xb = xloadp.tile([128, C], bf16, tag="xb")
                    nc.vector.tensor_copy(xb[:nsz], xf[:nsz])
                    for ci in range(NC_CH):
                        pt = ps_mm.tile([128, 128], bf16, tag="mm")
                        nc.tensor.transpose(
                            pt[:, :nsz], xb[:nsz, ci * 128:(ci + 1) * 128],
                            ident[:nsz, :nsz])
                        nc.any.tensor_copy(
                            xT[ci][:, noff:noff + nsz], pt[:, :nsz])

                # q,k per head in transposed orientation (M=96 chunks), with
                # bias extension rows at 96:128
                q_all = qkp.tile([KEXT, H, N], bf16, tag="q_all")
                k_all = qkp.tile([KEXT, H, N], bf16, tag="k_all")
                nc.vector.tensor_copy(
                    q_all[96:128, :, :],
                    wmaster[:, None, :].to_broadcast((32, H, N)))
                nc.vector.tensor_copy(
                    k_all[96:128, :, :],
                    umaster[:, None, :].to_broadcast((32, H, N)))
                cp_i = 0
                for oi in range(2 * C // 128):  # 12 chunks of q,k channels
                    ps = ps_mm.tile([128, 512], f32, tag="mm")
                    for ci in range(NC_CH):
                        nc.tensor.matmul(
                            ps[:, :N], qkvwT[ci][:, oi * 128:(oi + 1) * 128],
                            xT[ci][:],
                            start=(ci == 0), stop=(ci == NC_CH - 1))
                    t = (oi * 128) // C
                    dst = q_all if t == 0 else k_all
                    for (h, d_lo, d_hi, p_lo, p_hi) in head_fragments(
                            oi * 128, (oi + 1) * 128, t * C):
                        if cp_i % 3 == 0:
                            nc.vector.tensor_copy(dst[d_lo:d_hi, h, :],
                                                  ps[p_lo:p_hi, :N])
                        else:
                            nc.scalar.copy(dst[d_lo:d_hi, h, :],
                                           ps[p_lo:p_hi, :N])
                        cp_i += 1
                # stash original aux-key vectors, then overwrite aux-slot
                # columns with lang key vectors (the pre-bias "copy" edit)
                k_aux = qkp.tile([96, H, S2 - S1], bf16, tag="k_aux")
                nc.gpsimd.tensor_copy(k_aux[:, :, :], k_all[0:96, :, S2:N])
                nc.gpsimd.tensor_copy(k_all[0:96, :, S2:N],
                                      k_all[0:96, :, S1:S2])

                # v in natural orientation [n, 768] + [v | ones] per head
                v_sb = [vsbp.tile([128, C], bf16, name=f"vsb{nt}")
                        for nt in range(2)]
                vp = [vsbp.tile([128, H, D + 1], bf16, name=f"vp{nt}")
                      for nt in range(2)]
                for nt, (noff, nsz) in enumerate(NT):
                    for f0, fsz in [(0, 512), (512, 256)]:
                        ps = ps_mm.tile([128, 512], f32, tag="mm")
                        for ci in range(NC_CH):
                            nc.tensor.matmul(
                                ps[:nsz, :fsz],
                                xT[ci][:, noff:noff + nsz],
                                qkvwT[ci][:, 2 * C + f0:2 * C + f0 + fsz],
                                start=(ci == 0), stop=(ci == NC_CH - 1))
                        nc.any.tensor_copy(
                            v_sb[nt][:nsz, f0:f0 + fsz], ps[:nsz, :fsz])
                    nc.gpsimd.tensor_copy(
                        vp[nt][:nsz, :, 0:D],
                        v_sb[nt][:nsz, :].rearrange("p (h d) -> p h d", h=H))
                    nc.vector.memset(vp[nt][:nsz, :, D:D + 1], 1.0)
                # v_aux (tokens 216:236 = rows 88:108 of tile 2): misaligned
                # for compute engines -> extract via DMA, then pack
                va_stage = vsbp.tile([S2 - S1, C], bf16, tag="va_stage")
                nc.sync.dma_start(va_stage[:], v_sb[1][88:108, :])
                vap = vsbp.tile([S2 - S1, H, D + 1], bf16, tag="vap")
                nc.gpsimd.tensor_copy(
                    vap[:, :, 0:D],
                    va_stage[:, :].rearrange("p (h d) -> p h d", h=H))
                nc.vector.memset(vap[:, :, D:D + 1], 1.0)

                # attention, two heads at a time (shared psum tiles &
                # fused epilogue ops)
                aoT = aop.tile([96, H, N], bf16, tag="aoT")
                for hp in range(H // 2):
                    h0 = 2 * hp
                    p_sb = []
                    for jt, (joff, jsz) in enumerate(NT):
                        psj = ps_s.tile([128, 2, N], f32, tag="s")
                        for hh in range(2):
                            nc.tensor.matmul(
                                psj[:jsz, hh, :],
                                k_all[:, h0 + hh, joff:joff + jsz],
                                q_all[:, h0 + hh, :], start=True, stop=True,
                                skip_group_check=True)
                        pe = psbp.tile([128, 2, N], bf16, tag="p")
                        nc.scalar.activation(pe[:jsz], psj[:jsz],
                                             AF.Exp, scale=SCALE)
                        p_sb.append(pe)
                    # true aux x aux blocks (suppressed in main tiles)
                    ps_aa = ps_s.tile([S2 - S1, 2, S2 - S1], f32, tag="s")
                    for hh in range(2):
                        nc.tensor.matmul(ps_aa[:, hh, :],
                                         k_aux[:, h0 + hh, :],
                                         q_all[0:96, h0 + hh, S2:N],
                                         start=True, stop=True,
                                         skip_group_check=True)
                    p_aa = tinyp.tile([S2 - S1, 2, S2 - S1], bf16, tag="paa")
                    nc.scalar.activation(p_aa[:], ps_aa[:], AF.Exp,
                                         scale=SCALE)
                    # attn @ [v | ones] -> oT [D+1, q] (+ denominator row)
                    pso = ps_o.tile([D + 1, 2, N], f32, tag="o")
                    for hh in range(2):
                        for jt, (joff, jsz) in enumerate(NT):
                            nc.tensor.matmul(pso[:, hh, :],
                                             vp[jt][:jsz, h0 + hh, :],
                                             p_sb[jt][:jsz, hh, :],
                                             start=(jt == 0), stop=False,
                                             skip_group_check=True)
                        nc.tensor.matmul(pso[:, hh, S2:N],
                                         vap[:, h0 + hh, :], p_aa[:, hh, :],
                                         start=False, stop=True,
                                         skip_group_check=True)
                    # normalize both heads in fused ops
                    den = tinyp.tile([1, 2, N], f32, tag="den")
                    nc.vector.tensor_copy(den[:], pso[D:D + 1, :, :])
                    r_f = tinyp.tile([1, 2, N], f32, tag="rf")
                    nc.vector.reciprocal_approx_fast(r_f[:], den[:])
                    rbc = psbp.tile([128, 2, N], f32, tag="rbc")
                    nc.gpsimd.partition_broadcast(
                        rbc[:], r_f[0:1, :, :].rearrange("p a b -> p (a b)"))
                    nc.vector.tensor_tensor(
                        aoT[:, h0:h0 + 2, :], pso[0:D, :, :], rbc[0:D, :, :],
                        OP.mult)

                # proj + bias + store (contract per head, K=96)
                for nt, (noff, nsz) in enumerate(NT):
                    osb = osbp.tile([128, C], f32, tag="osb")
                    for f0, fsz in [(0, 512), (512, 256)]:
                        ps = ps_mm.tile([128, 512], f32, tag="mm")
                        for h in range(H):
                            nc.tensor.matmul(
                                ps[:nsz, :fsz],
                                aoT[:, h, noff:noff + nsz],
                                projwTh[h][:, f0:f0 + fsz],
                                start=(h == 0), stop=(h == H - 1))
                        nc.vector.tensor_tensor(
                            osb[:nsz, f0:f0 + fsz], ps[:nsz, :fsz],
                            pb_bcast[:nsz, f0:f0 + fsz], OP.add)
                    nc.sync.dma_start(out_d[b, noff:noff + nsz, :],
                                      osb[:nsz])

    nc.compile()
    return nc


_NC_CACHE = {}


def _get_nc(b_loc):
    if b_loc not in _NC_CACHE:
        _NC_CACHE[b_loc] = build(b_loc)
    return _NC_CACHE[b_loc]


def _run(inputs, trace=False):
    from concourse.bass_utils import run_bass_kernel_spmd

    x = np.ascontiguousarray(np.asarray(inputs["x"], dtype=np.float32))
    qkv_w = np.ascontiguousarray(np.asarray(inputs["qkv_w"], dtype=np.float32))
    proj_w = np.ascontiguousarray(np.asarray(inputs["proj_w"], dtype=np.float32))
    proj_b = np.ascontiguousarray(np.asarray(inputs["proj_b"], dtype=np.float32))

    nc = _get_nc(B_LOC)
    in_maps = [
        {
            "x": np.ascontiguousarray(x[i * B_LOC:(i + 1) * B_LOC]),
            "qkv_w": qkv_w,
            "proj_w": proj_w,
            "proj_b": proj_b,
        }
        for i in range(N_CORES)
    ]
    res = run_bass_kernel_spmd(
        nc, in_maps, core_ids=list(range(N_CORES)), trace=trace)
    out = np.concatenate([r["out"] for r in res.results], axis=0)
    return out, res


def kernel(x, qkv_w, proj_w, proj_b):
    out, _ = _run({"x": x, "qkv_w": qkv_w, "proj_w": proj_w,
                   "proj_b": proj_b})
    return out
